# revision 1
# baseline (speedup 1.0000x reference)
import numpy as np
import jax
import jax.numpy as jnp

# nn_CausalLinearAttention: query (8, 512, 64, 128) f32; W* (128,128); b* (128,)
# Data-parallel over batch B=8 -> one batch element per NeuronCore (8 cores).
# Per core: chunked causal linear attention (fast_transformers style),
# feature map phi(x) = elu(x)+1, eps = 1e-6.

HEADS = 8
HEAD_DIM = 16
EPS = 1e-6
L = 512
N = 64
F = 128
C = 128          # time chunk
NC = L // C      # 4 chunks


def _per_device(xb, Wq, bq, Wk, bk, Wv, bv):
    # xb: (L, N, F) one batch element
    x = jnp.swapaxes(xb, 0, 1)                    # (N, L, F)
    q = jax.nn.elu(x @ Wq + bq) + 1.0             # (N, L, 128)
    k = jax.nn.elu(x @ Wk + bk) + 1.0
    v = x @ Wv + bv
    H, E = HEADS, HEAD_DIM
    qc = q.reshape(N, NC, C, H, E)
    kc = k.reshape(N, NC, C, H, E)
    vc = v.reshape(N, NC, C, H, E)

    # intra-chunk (diagonal blocks), causal mask incl. diagonal
    A = jnp.einsum('ncthe,ncshe->nchts', qc, kc)          # (N,NC,H,C,C)
    mask = jnp.tril(jnp.ones((C, C), dtype=x.dtype))
    Am = A * mask
    intra = jnp.einsum('nchts,ncshf->ncthf', Am, vc)      # (N,NC,C,H,E)
    den_intra = jnp.sum(Am, axis=-1)                      # (N,NC,H,C)
    den_intra = jnp.moveaxis(den_intra, 2, 3)             # (N,NC,C,H)

    # inter-chunk via exclusive cumulative KV state
    kv = jnp.einsum('ncshe,ncshf->nchef', kc, vc)         # (N,NC,H,E,E)
    S = jnp.cumsum(kv, axis=1) - kv                       # exclusive prefix
    inter = jnp.einsum('ncthe,nchef->ncthf', qc, S)       # (N,NC,C,H,E)

    ks = jnp.sum(kc, axis=2)                              # (N,NC,H,E)
    Ks = jnp.cumsum(ks, axis=1) - ks                      # exclusive prefix
    den_inter = jnp.einsum('ncthe,nche->ncth', qc, Ks)    # (N,NC,C,H)

    den = den_intra + den_inter + EPS                     # (N,NC,C,H)
    out = (intra + inter) / den[..., None]                # (N,NC,C,H,E)
    out = out.reshape(N, L, H * E)
    return jnp.swapaxes(out, 0, 1)                        # (L, N, 128)


_pmapped = None


def _get_pmapped():
    global _pmapped
    if _pmapped is None:
        _pmapped = jax.pmap(
            _per_device,
            in_axes=(0, None, None, None, None, None, None),
            devices=jax.devices()[:8],
        )
    return _pmapped


def kernel(query, Wq, bq, Wk, bk, Wv, bv):
    fn = _get_pmapped()
    out = fn(
        jnp.asarray(query, jnp.float32),
        jnp.asarray(Wq, jnp.float32), jnp.asarray(bq, jnp.float32),
        jnp.asarray(Wk, jnp.float32), jnp.asarray(bk, jnp.float32),
        jnp.asarray(Wv, jnp.float32), jnp.asarray(bv, jnp.float32),
    )
    return np.asarray(out, dtype=np.float32)



# revision 12
# speedup vs baseline: 9282.9325x; 9282.9325x over previous
"""Causal linear attention (fast_transformers style) on 8 Trainium2 cores.

query (8, 512, 64, 128) f32; Wq/Wk/Wv (128,128); bq/bk/bv (128,).
Data-parallel over batch B=8 -> one batch element per NeuronCore.

Per core (Bass/Tile kernel):
  x^T per node sequence arrives feat-major (F=128 partitions, L=512).
  qT = Wq.T @ xT, kT = Wk.T @ xT  (feat-major, PSUM)
  phi(x) = elu(x)+1 = exp(min(x,0)) + relu(x)   (ACT Exp/Relu + DVE)
  chunked causal attention, chunk C=128:
    AT_h = phi_k_h @ phi_q_h^T        (s,t) per head, via K=64 strips with
                                      per-partition head-select masks on q
    ATm = AT * triu(s<=t)             (DVE PSUM->SBUF move, bf16)
    intra_h = ATm_h.T @ [v_h|1]       -> out PSUM (t, 17h+f), start group
    inter   = qT_c.T @ S_bd           -> accumulate (exclusive KV state)
    S      += kt_c.T.T @ [v|1]        (kt_c = PE-transposed phi_k chunk)
    out     = num * recip(den + eps)  -> DMA to DRAM t-major
"""

import numpy as np
from concurrent.futures import ThreadPoolExecutor

HEADS = 8
E = 16
L = 512
NSEQ = 64
F = 128
C = 128
NCHUNK = L // C
EPS = 1e-6
NCORES = 8
VX = HEADS * (E + 1)  # 136: per-head [v (16) | ones] column blocks


def build_nc(nseq=NSEQ):
    import concourse.bass as bass
    import concourse.bacc as bacc
    import concourse.tile as tile
    from concourse import mybir

    dt = mybir.dt
    nc = bacc.Bacc("TRN2", target_bir_lowering=False, debug=False)

    xT = nc.dram_tensor("xT", [nseq, F, L], dt.bfloat16, kind="ExternalInput")
    wq = nc.dram_tensor("wq", [F, F], dt.bfloat16, kind="ExternalInput")
    wk = nc.dram_tensor("wk", [F, F], dt.bfloat16, kind="ExternalInput")
    wv = nc.dram_tensor("wv", [F, F], dt.bfloat16, kind="ExternalInput")
    bq = nc.dram_tensor("bq", [F, 1], dt.float32, kind="ExternalInput")
    bk = nc.dram_tensor("bk", [F, 1], dt.float32, kind="ExternalInput")
    bvb = nc.dram_tensor("bvb", [F, F], dt.bfloat16, kind="ExternalInput")
    triu = nc.dram_tensor("triu", [C, HEADS * C], dt.bfloat16, kind="ExternalInput")
    bdiag = nc.dram_tensor("bdiag", [F, VX], dt.bfloat16, kind="ExternalInput")
    qmask = nc.dram_tensor("qmask", [F, 4], dt.float32, kind="ExternalInput")
    ident = nc.dram_tensor("ident", [F, F], dt.bfloat16, kind="ExternalInput")
    y = nc.dram_tensor("y", [L, nseq, F], dt.bfloat16, kind="ExternalOutput")

    with tile.TileContext(nc) as tc:
        with (
            tc.tile_pool(name="consts", bufs=1) as consts,
            tc.tile_pool(name="xp", bufs=2) as xp,
            tc.tile_pool(name="phip", bufs=2) as phip,
            tc.tile_pool(name="tmp", bufs=2) as tmp,
            tc.tile_pool(name="vexp", bufs=2) as vexp,
            tc.tile_pool(name="ktp", bufs=2) as ktp,
            tc.tile_pool(name="atm", bufs=2) as atmp,
            tc.tile_pool(name="outp", bufs=2) as outp,
            tc.tile_pool(name="ps_q", bufs=1, space="PSUM") as ps_q,
            tc.tile_pool(name="ps_k", bufs=1, space="PSUM") as ps_k,
            tc.tile_pool(name="ps_a", bufs=1, space="PSUM") as ps_a,
            tc.tile_pool(name="ps_o", bufs=1, space="PSUM") as ps_o,
            tc.tile_pool(name="ps_s", bufs=1, space="PSUM") as ps_s,
            tc.tile_pool(name="ps_v", bufs=1, space="PSUM") as ps_v,
            tc.tile_pool(name="ps_t", bufs=1, space="PSUM") as ps_t,
        ):
            wq_t = consts.tile([F, F], dt.bfloat16)
            nc.sync.dma_start(wq_t[:], wq[:])
            wk_t = consts.tile([F, F], dt.bfloat16)
            nc.sync.dma_start(wk_t[:], wk[:])
            wv_t = consts.tile([F, F], dt.bfloat16)
            nc.sync.dma_start(wv_t[:], wv[:])
            bq_t = consts.tile([F, 1], dt.float32)
            nc.sync.dma_start(bq_t[:], bq[:])
            bk_t = consts.tile([F, 1], dt.float32)
            nc.sync.dma_start(bk_t[:], bk[:])
            bvb_t = consts.tile([F, F], dt.bfloat16)
            nc.sync.dma_start(bvb_t[:], bvb[:])
            triu_t = consts.tile([C, HEADS * C], dt.bfloat16)
            nc.sync.dma_start(triu_t[:], triu[:])
            bdiag_t = consts.tile([F, VX], dt.bfloat16)
            nc.sync.dma_start(bdiag_t[:], bdiag[:])
            qm_t = consts.tile([F, 4], dt.float32)
            nc.sync.dma_start(qm_t[:], qmask[:])
            id_t = consts.tile([F, F], dt.bfloat16)
            nc.sync.dma_start(id_t[:], ident[:])

            for n in range(nseq):
                x_t = xp.tile([F, L], dt.bfloat16)
                nc.sync.dma_start(x_t[:], xT[n])

                # ---- projections (feat-major) ----
                q_ps = ps_q.tile([F, L], dt.float32)
                nc.tensor.matmul(q_ps[:], wq_t[:], x_t[:], start=True, stop=True)
                k_ps = ps_k.tile([F, L], dt.float32)
                nc.tensor.matmul(k_ps[:], wk_t[:], x_t[:], start=True, stop=True)

                # ---- phi = exp(min(x+b,0)) + relu(x+b) ----
                def phi(ps, bias_t, name):
                    m = tmp.tile([F, L], dt.bfloat16, tag="m")
                    nc.vector.tensor_scalar(
                        m[:], ps[:], bias_t[:], 0.0,
                        op0=mybir.AluOpType.add, op1=mybir.AluOpType.min)
                    e = tmp.tile([F, L], dt.bfloat16, tag="e")
                    nc.scalar.activation(
                        e[:], m[:], mybir.ActivationFunctionType.Exp)
                    r = tmp.tile([F, L], dt.bfloat16, tag="r")
                    nc.scalar.activation(
                        r[:], ps[:], mybir.ActivationFunctionType.Relu,
                        bias=bias_t[:], scale=1.0)
                    ph = phip.tile([F, L], dt.bfloat16, tag=name)
                    nc.vector.tensor_add(ph[:], e[:], r[:])
                    return ph

                phq = phi(q_ps, bq_t, "phq")
                phk = phi(k_ps, bk_t, "phk")

                # head-select masked copies of phi_q (for K=64 strip matmuls)
                phqm = []
                for j in range(4):
                    pm = phip.tile([F, L], dt.bfloat16, tag=f"phqm{j}")
                    nc.vector.tensor_scalar(
                        pm[:], phq[:], qm_t[:, j:j + 1], None,
                        op0=mybir.AluOpType.mult)
                    phqm.append(pm)

                # ---- per chunk: v, kt (t-major), attention ----
                vex = []
                kts = []
                for c in range(NCHUNK):
                    cs = slice(c * C, (c + 1) * C)
                    v_ps = ps_v.tile([C, F], dt.float32)
                    nc.tensor.matmul(v_ps[:], x_t[:, cs], wv_t[:],
                                     start=True, stop=True)
                    vx = vexp.tile([C, VX], dt.bfloat16, tag=f"vex{c}")
                    nc.vector.memset(vx[:, E::E + 1], 1.0)
                    # vex[:, 17h+f] = v_ps[:, 16h+f] + bv  (strided dest)
                    dst = vx[:].rearrange("p (h x) -> p h x", h=HEADS)[:, :, 0:E]
                    src = v_ps[:].rearrange("p (h x) -> p h x", h=HEADS)
                    bsr = bvb_t[:].rearrange("p (h x) -> p h x", h=HEADS)
                    nc.vector.tensor_add(dst, src, bsr)
                    vex.append(vx)

                    kt_ps = ps_t.tile([C, F], dt.bfloat16)
                    nc.tensor.transpose(kt_ps[:], phk[:, cs], id_t[:])
                    kt = ktp.tile([C, F], dt.bfloat16, tag=f"kt{c}")
                    nc.scalar.activation(
                        kt[:], kt_ps[:], mybir.ActivationFunctionType.Copy)
                    kts.append(kt)

                S_ps = ps_s.tile([F, VX], dt.float32)

                for c in range(NCHUNK):
                    cs = slice(c * C, (c + 1) * C)
                    # scores AT_h (s,t), K=64 strips x 4 head-masks
                    a_ps = ps_a.tile([C, HEADS * C], dt.float32)
                    for h in range(HEADS):
                        half, j = divmod(h, 4)
                        rows = slice(64 * half, 64 * (half + 1))
                        nc.tensor.matmul(
                            a_ps[:, h * C:(h + 1) * C],
                            phk[rows, cs], phqm[j][rows, cs],
                            start=True, stop=True)
                    atm = atmp.tile([C, HEADS * C], dt.bfloat16)
                    nc.vector.tensor_mul(atm[:], a_ps[:], triu_t[:])

                    # output accumulator (t, [num16|den1] x 8)
                    o_ps = ps_o.tile([C, VX], dt.float32)
                    for h in range(HEADS):
                        nc.tensor.matmul(
                            o_ps[:, h * (E + 1):(h + 1) * (E + 1)],
                            atm[:, h * C:(h + 1) * C], vex[c][:, h * (E + 1):(h + 1) * (E + 1)],
                            start=(h == 0), stop=(c == 0 and h == HEADS - 1))
                    if c > 0:
                        sbd = tmp.tile([F, VX], dt.bfloat16, tag="sbd")
                        nc.vector.tensor_mul(sbd[:], S_ps[:], bdiag_t[:])
                        nc.tensor.matmul(o_ps[:], phq[:, cs], sbd[:],
                                         start=False, stop=True)
                    # state += kv (after exclusive copy above); the last
                    # chunk's update is never read -> skip it
                    if c < NCHUNK - 1:
                        nc.tensor.matmul(S_ps[:], kts[c][:], vex[c][:],
                                         start=(c == 0), stop=True,
                                         skip_group_check=(c > 0))

                    # normalize: out = num / (den + eps)
                    o3 = o_ps[:].rearrange("p (h x) -> p h x", h=HEADS)
                    den = tmp.tile([C, HEADS], dt.float32, tag="den")
                    nc.vector.tensor_scalar(
                        den[:], o3[:, :, E], EPS, None, op0=mybir.AluOpType.add)
                    rin = tmp.tile([C, HEADS], dt.float32, tag="rin")
                    nc.vector.reciprocal(rin[:], den[:])
                    o_sb = outp.tile([C, F], dt.bfloat16)
                    o_sb3 = o_sb[:].rearrange("p (h x) -> p h x", h=HEADS)
                    rin3 = rin[:][:, :, None].broadcast_to([C, HEADS, E])
                    nc.vector.tensor_mul(o_sb3, o3[:, :, 0:E], rin3)
                    nc.sync.dma_start(y[cs, n, :], o_sb[:])
    nc.finalize()
    return nc


_BF16 = None


def _bf16():
    global _BF16
    if _BF16 is None:
        import ml_dtypes
        _BF16 = ml_dtypes.bfloat16
    return _BF16


def _consts(Wq, bq, Wk, bk, Wv, bv):
    bf16 = _bf16()
    tri = np.triu(np.ones((C, C), np.float32))  # AT[s,t] keep s<=t
    triu_rep = np.tile(tri, (1, HEADS)).astype(bf16)
    bd = np.zeros((F, VX), np.float32)
    for h in range(HEADS):
        bd[h * E:(h + 1) * E, h * (E + 1):(h + 1) * (E + 1)] = 1.0
    qm = np.zeros((F, 4), np.float32)
    for p in range(F):
        qm[p, (p % 64) // E] = 1.0
    return {
        "wq": np.ascontiguousarray(Wq.astype(bf16)),
        "wk": np.ascontiguousarray(Wk.astype(bf16)),
        "wv": np.ascontiguousarray(Wv.astype(bf16)),
        "bq": np.ascontiguousarray(bq.reshape(F, 1).astype(np.float32)),
        "bk": np.ascontiguousarray(bk.reshape(F, 1).astype(np.float32)),
        "bvb": np.ascontiguousarray(
            np.broadcast_to(bv.astype(np.float32), (F, F)).astype(bf16)),
        "triu": triu_rep,
        "bdiag": bd.astype(bf16),
        "qmask": qm,
        "ident": np.eye(F, dtype=bf16),
    }


_NC = None
TRACE = False
TRACE_DIR = None
LAST_RESULT = None


def kernel(query, Wq, bq, Wk, bk, Wv, bv):
    global _NC, LAST_RESULT
    from concourse import bass_utils

    bf16 = _bf16()
    consts = _consts(Wq, bq, Wk, bk, Wv, bv)

    def prep(c):
        # (L, N, F) -> (N, F, L) bf16, feat-major per sequence
        return np.ascontiguousarray(
            np.asarray(query[c]).transpose(1, 2, 0)).astype(bf16)

    with ThreadPoolExecutor(NCORES) as ex:
        xts = list(ex.map(prep, range(NCORES)))

    in_maps = [dict(consts, xT=xts[c]) for c in range(NCORES)]

    if _NC is None:
        _NC = build_nc()
    kwargs = {}
    if TRACE:
        kwargs = dict(trace=True, tmpdir=TRACE_DIR)
    res = bass_utils.run_bass_kernel_spmd(
        _NC, in_maps, core_ids=list(range(NCORES)), **kwargs)
    LAST_RESULT = res
    out = np.empty((NCORES, L, NSEQ, F), np.float32)
    for c in range(NCORES):
        out[c] = res.results[c]["y"].astype(np.float32)
    return out
